# revision 42
# baseline (speedup 1.0000x reference)
"""Bass/Trainium2 kernel for nn_EquivariantPosUpdate — 8-core edge-parallel, v2.

Per core: 1024 edges in 8 tiles of 128 (edges on partitions).
Key design vs v1 (1.00 ms -> 0.50 ms on-device):
  - all matmuls in fp16 (fp32 matmul = 4 cy/row + LOW_HIGH double-issue;
    fp16 = 1 cy/row and 8x the mantissa of bf16 -> rel err 1.7e-3)
  - node features gathered per edge on HOST (pure data staging); no phase A,
    no indirect DMAs; all per-edge inputs staged to SBUF in one DMA each
  - radial-MLP stages phased across tiles so the Scalar engine loads each
    activation table once per stage (Exp/Sqrt/Sigmoid): 13 table loads
    total instead of ~70 (1.3 us each)
  - depthwise-TP weight chunks: PE matmul (fp16) -> Scalar evac to fp16 SBUF
    (DVE reads from PSUM are ~3x slower than SBUF) -> ss-multiplies on
    GpSimd, everything else mult+grouped-reduce on DVE (the span limiter)
  - adaLN time-mod table gathered per edge via one-hot matmul (no DRAM trip)
  - scatter: edges sorted by src on host; each core covers a 384-node window;
    per-tile one-hot matmuls only over the 1-3 chunks the tile touches
    (chunk ranges specialized at build time from the actual edge_index)
"""
import sys, os
sys.path.insert(0, '/opt/trn_rl_repo')
import numpy as np
import ml_dtypes
from contextlib import ExitStack

import concourse.bass as bass
import concourse.bacc as bacc
import concourse.mybir as mybir
import concourse.tile as tile
from concourse.bass import AP
from concourse.masks import make_identity

F32 = mybir.dt.float32
BF16 = mybir.dt.float16  # 2-byte; fp16 for precision (same PE/DVE speed)
AX = mybir.AxisListType
OP = mybir.AluOpType
ACTF = mybir.ActivationFunctionType
BF = np.float16

N, E, G, NB = 2048, 8192, 64, 128
NC_CORES = 8
EC = E // NC_CORES          # 1024
P = 128
T = EC // P                 # 8 tiles
M0, M1 = 64, 32
S_TP = 96
CUTOFF = 5.0
DEBUG = False
STAGE = int(os.environ.get('K2STAGE', '99'))

# ---- replicated constant rows ----
ROWSF = {}
_o = 0
for _n, _w in [('A', 128), ('B', 128), ('sp_b2', 1), ('eps', 1)]:
    ROWSF[_n] = (_o, _w); _o += _w
RWF = _o
ROWSB = {}
_o = 0
for _n, _w in [('g1p', 128), ('b1p', 128), ('g2p', 128), ('b2p', 128),
               ('sbs', 64), ('dbs', 64), ('nt_bs', 64), ('et_bs', 64),
               ('nf_bias', 96), ('ef_bias', 96), ('sp_b1', 32),
               ('spW2r', 32), ('normbt', 192)]:
    ROWSB[_n] = (_o, _w); _o += _w
RWB = _o


def rsl(rep, rows, name, nrows=P):
    off, w = rows[name]
    return rep[0:nrows, off:off + w]


def ap3(t, dims, offset=0):
    base = t[:, :] if not isinstance(t, AP) else t
    return AP(base.tensor, base.offset + offset,
              [base.ap[0]] + [list(d) for d in dims])


def build_nc(CHL, tile_chunks):
    """CHL: local node chunks per core; tile_chunks: [(lo,hi)] per tile."""
    nc = bacc.Bacc("TRN2", target_bir_lowering=False, debug=False,
                   num_devices=NC_CORES)
    Tn = {}

    def din(name, shape, dtype=BF16):
        Tn[name] = nc.dram_tensor(name, shape, dtype, kind="ExternalInput")
        return Tn[name]

    din('W3nf', [64, 10240]); din('W3ef', [64, 5120])
    din('W1p', [128, 128]); din('W2blk', [128, 128])
    din('srcWs', [128, 64]); din('dstWs', [128, 64])
    din('srcWv', [64, 32]); din('dstWv', [64, 32])
    din('ntWs', [96, 64]); din('ntWv', [128, 32])
    din('etWs', [64, 64]); din('etWv', [32, 32])
    din('spW1', [96, 32]); din('normWt', [128, 192]); din('tT', [128, G])
    din('hsT', [320, EC]); din('hdT', [320, EC]); din('heT', [160, EC])
    din('rowsf', [1, RWF], F32); din('rowsb', [1, RWB])
    din('edf', [P, T * 8], F32); din('gidr', [1, EC], F32)
    outp = nc.dram_tensor('outp', [CHL * P, 3], F32, kind="ExternalOutput")
    Tn['outp'] = outp
    if DEBUG:
        for nm, sh in [('dbg_fs', [EC, 96]), ('dbg_as', [EC, 96]),
                       ('dbg_force', [EC, 3]), ('dbg_h2', [EC, 128]),
                       ('dbg_sn', [EC, 96]), ('dbg_fv', [EC, 384]),
                       ('dbg_s1', [EC, 64]), ('dbg_v1', [EC, 96]),
                       ('dbg_es', [EC, 64]), ('dbg_ns', [EC, 64]),
                       ('dbg_nv', [EC, 96])]:
            Tn[nm] = nc.dram_tensor(nm, sh, F32, kind="ExternalOutput")

    with tile.TileContext(nc) as tc:
        with ExitStack() as ctx:
            with nc.allow_low_precision(reason="bf16 pipeline; rel-err gate 2e-2"):
                _build(ctx, tc, nc, Tn, CHL, tile_chunks)
    nc.compile()
    return nc


def _build(ctx, tc, nc, Tn, CHL, tile_chunks):
    consts = ctx.enter_context(tc.tile_pool(name="consts", bufs=1))
    ph = ctx.enter_context(tc.tile_pool(name="ph", bufs=1))      # per-tile persist
    sb = ctx.enter_context(tc.tile_pool(name="sb", bufs=4))      # transient
    sbq = ctx.enter_context(tc.tile_pool(name="sbq", bufs=4))    # dtp transient
    ps = ctx.enter_context(tc.tile_pool(name="ps", bufs=2, space="PSUM"))
    psw = ctx.enter_context(tc.tile_pool(name="psw", bufs=2, space="PSUM"))
    psx = ctx.enter_context(tc.tile_pool(name="psx", bufs=1, space="PSUM"))
    dma = nc.sync.dma_start

    def load(name, pool=consts):
        t = pool.tile(Tn[name].shape, Tn[name].dtype, tag="ld_" + name,
                      name="ld_" + name)
        dma(t[:], Tn[name][:])
        return t

    # ---------------- setup ----------------
    # DMA order = need order: per-edge inputs + first-stage weights first,
    # the big W3 tables (only needed ~100us in, at the dtp stage) last.
    edf = load('edf'); gidr = load('gidr')
    rowsf1 = load('rowsf'); rowsb1 = load('rowsb')
    W1p = load('W1p'); W2blk = load('W2blk')
    srcWs = load('srcWs'); dstWs = load('dstWs')
    srcWv = load('srcWv'); dstWv = load('dstWv')
    ntWs = load('ntWs'); ntWv = load('ntWv')
    etWs = load('etWs'); etWv = load('etWv')
    spW1 = load('spW1'); normWt = load('normWt'); tT = load('tT')
    heS = consts.tile([64, EC], BF16)
    dma(heS[:], Tn['heT'][0:64, :])
    heV = [consts.tile([32, EC], BF16, tag=f"heV{x}", name=f"heV{x}")
           for x in range(3)]
    for x in range(3):
        dma(heV[x][:], Tn['heT'][64 + 32 * x:96 + 32 * x, :])
    hsS = consts.tile([128, EC], BF16)
    dma(hsS[:], Tn['hsT'][0:128, :])
    hdS = consts.tile([128, EC], BF16)
    dma(hdS[:], Tn['hdT'][0:128, :])
    hsV = [consts.tile([64, EC], BF16, tag=f"hsV{x}", name=f"hsV{x}")
           for x in range(3)]
    hdV = [consts.tile([64, EC], BF16, tag=f"hdV{x}", name=f"hdV{x}")
           for x in range(3)]
    for x in range(3):
        dma(hsV[x][:], Tn['hsT'][128 + 64 * x:192 + 64 * x, :])
        dma(hdV[x][:], Tn['hdT'][128 + 64 * x:192 + 64 * x, :])
    W3nf = load('W3nf'); W3ef = load('W3ef')

    repf = consts.tile([P, RWF], F32)
    nc.gpsimd.partition_broadcast(repf[:], rowsf1[:])
    repb = consts.tile([P, RWB], BF16)
    nc.gpsimd.partition_broadcast(repb[:], rowsb1[:])

    ident = consts.tile([P, P], BF16)
    make_identity(nc, ident[:])
    iota_i = consts.tile([P, P], mybir.dt.int32)
    nc.gpsimd.iota(iota_i[:], pattern=[[1, P]], base=0, channel_multiplier=0)
    iota_bf = consts.tile([P, P], BF16)
    nc.vector.tensor_copy(iota_bf[:], iota_i[:])
    iotap_i = consts.tile([64, 1], mybir.dt.int32)
    nc.gpsimd.iota(iotap_i[:], pattern=[[1, 1]], base=0, channel_multiplier=1)
    iotap_bf = consts.tile([64, 1], BF16)
    nc.vector.tensor_copy(iotap_bf[:], iotap_i[:])

    # time-mod table [G, 192] = t @ normWt + normbt (scale half has +1 folded)
    md_ps = ps.tile([G, 192], F32, tag="ps_sm")
    nc.tensor.matmul(md_ps[:], tT[:], normWt[:], start=True, stop=True)
    modtab = consts.tile([G, 192], BF16)
    nc.vector.tensor_tensor(modtab[:], md_ps[:], rsl(repb, ROWSB, 'normbt', G),
                            op=OP.add)

    acc_sb = consts.tile([P, CHL * 3], F32)
    nc.vector.memset(acc_sb[:], 0.0)

    # per-tile persistent tiles
    def pht(name, t, shape, dtype=BF16):
        return ph.tile(shape, dtype, tag=f"{name}{t}", name=f"{name}{t}")

    S1 = {}; V1 = {}; S2 = {}; V2 = {}; ES = {}; EV = {}
    ESR = {}; ESRT = {}; CEN1 = {}; RST1 = {}; H1 = {}; H1T = {}
    CEN2 = {}; RST2 = {}; H2 = {}; H2TN = {}; H2TE = {}
    ZSQ = {}; VAR1 = {}; VAR2 = {}; VARA = {}
    FS = {}; FV = {}; NS = {}; NV = {}
    AS = {}; CENA = {}; RSTA = {}; SN = {}; HD = {}; HDS = {}
    FORCE = {}; MODPS = {}

    def tcols(t):
        return slice(t * P, (t + 1) * P)

    def ecol(t, j):
        return edf[:, 8 * t + j:8 * t + j + 1]

    def _finish():
        for ch in range(CHL):
            dma(Tn['outp'][ch * P:(ch + 1) * P, :], acc_sb[:, 3 * ch:3 * ch + 3])

    if STAGE < 2:
        _finish(); return
    # ============ projections: s1/v1 (src), s2/v2 (dst), es/ev (edge) ========
    for t in range(T):
        s1p = ps.tile([P, 64], F32, tag="ps_sm")
        nc.tensor.matmul(s1p[:], hsS[:, tcols(t)], srcWs[:], start=True, stop=True)
        S1[t] = pht('s1', t, [P, 64])
        nc.vector.tensor_tensor(S1[t][:], s1p[:], rsl(repb, ROWSB, 'sbs'), op=OP.add)
        s2p = ps.tile([P, 64], F32, tag="ps_sm")
        nc.tensor.matmul(s2p[:], hdS[:, tcols(t)], dstWs[:], start=True, stop=True)
        S2[t] = pht('s2', t, [P, 64])
        nc.vector.tensor_tensor(S2[t][:], s2p[:], rsl(repb, ROWSB, 'dbs'), op=OP.add)
        V1[t] = pht('v1', t, [P, 96])
        V2[t] = pht('v2', t, [P, 96])
        for x in range(3):
            vp = ps.tile([P, 32], F32, tag="ps_sm")
            nc.tensor.matmul(vp[:], hsV[x][:, tcols(t)], srcWv[:], start=True,
                             stop=True)
            nc.scalar.copy(V1[t][:, 32 * x:32 * x + 32], vp[:])
            vp2 = ps.tile([P, 32], F32, tag="ps_sm")
            nc.tensor.matmul(vp2[:], hdV[x][:, tcols(t)], dstWv[:], start=True,
                             stop=True)
            nc.scalar.copy(V2[t][:, 32 * x:32 * x + 32], vp2[:])
        esp = ps.tile([P, 64], F32, tag="ps_sm")
        nc.tensor.matmul(esp[:], heS[:, tcols(t)], etWs[:], start=True, stop=True)
        ES[t] = pht('es', t, [P, 64])
        nc.vector.tensor_tensor(ES[t][:], esp[:], rsl(repb, ROWSB, 'et_bs'), op=OP.add)
        EV[t] = pht('ev', t, [P, 96])
        for x in range(3):
            evp = ps.tile([P, 32], F32, tag="ps_sm")
            nc.tensor.matmul(evp[:], heV[x][:, tcols(t)], etWv[:], start=True,
                             stop=True)
            nc.scalar.copy(EV[t][:, 32 * x:32 * x + 32], evp[:])

    if STAGE < 3:
        _finish(); return
    # ============ RBF ============
    for t in range(T):
        z = sb.tile([P, NB], F32, tag="z")
        nc.vector.scalar_tensor_tensor(z[:], rsl(repf, ROWSF, 'A'),
                                       ecol(t, 0), rsl(repf, ROWSF, 'B'),
                                       op0=OP.mult, op1=OP.add)
        ZSQ[t] = pht('zsq', t, [P, NB], F32)
        nc.vector.tensor_mul(ZSQ[t][:], z[:], z[:])
    for t in range(T):
        ESR[t] = pht('esr', t, [P, NB])
        nc.scalar.activation(ESR[t][:], ZSQ[t][:], ACTF.Exp, scale=-0.5)
    for t in range(T):
        ep = ps.tile([NB, P], BF16, tag="ps_tp")
        nc.tensor.transpose(ep[:], ESR[t][:], ident[:])
        ESRT[t] = pht('esrT', t, [NB, P])
        nc.scalar.copy(ESRT[t][:], ep[:])

    if STAGE < 4:
        _finish(); return
    # ============ radial layer 1 ============
    x1_all = psx.tile([P, T * 128], F32, tag="x1_all")
    for t in range(T):
        nc.tensor.matmul(x1_all[:, t * 128:(t + 1) * 128], ESRT[t][:], W1p[:],
                         start=True, stop=True, skip_group_check=True)

    def ln_pair(t, x_ps, CEN, VAR, tag):
        """joint LN over two 64-groups; fills CEN/VAR."""
        mu = sb.tile([P, 2], F32, tag=f"mu{tag}")
        nc.vector.tensor_reduce(mu[:], ap3(x_ps, [[64, 2], [1, 64]]),
                                axis=AX.X, op=OP.add)
        nc.vector.tensor_scalar_mul(mu[:], mu[:], 1.0 / 64)
        CEN[t] = pht(f'cen{tag}', t, [P, 128], F32)
        nc.vector.tensor_tensor(CEN[t][:], x_ps, ap3(mu, [[1, 2], [0, 64]]),
                                op=OP.subtract)
        sq = sb.tile([P, 128], F32, tag=f"sq{tag}")
        nc.vector.tensor_mul(sq[:], CEN[t][:], CEN[t][:])
        VAR[t] = pht(f'var{tag}', t, [P, 2], F32)
        nc.vector.tensor_reduce(VAR[t][:], ap3(sq, [[64, 2], [1, 64]]),
                                axis=AX.X, op=OP.add)

    def ln_rsqrt(t, VAR, RST, tag):
        std = pht(f'std{tag}', t, [P, 2], F32)
        nc.scalar.activation(std[:], VAR[t][:], ACTF.Sqrt, scale=1.0 / 64,
                             bias=repf[:, ROWSF['eps'][0]:ROWSF['eps'][0] + 1])
        RST[t] = pht(f'rst{tag}', t, [P, 2], F32)
        nc.vector.reciprocal(RST[t][:], std[:])

    def ln_apply(t, CEN, RST, H, tag, gname, bname):
        t1 = sb.tile([P, 128], BF16, tag=f"t1{tag}")
        nc.gpsimd.tensor_tensor(t1[:], CEN[t][:],
                                ap3(RST[t], [[1, 2], [0, 64]]), op=OP.mult)
        t2 = sb.tile([P, 128], BF16, tag=f"t2{tag}")
        nc.gpsimd.tensor_tensor(t2[:], t1[:], rsl(repb, ROWSB, gname), op=OP.mult)
        H[t] = pht(f'hln{tag}', t, [P, 128])
        nc.gpsimd.tensor_tensor(H[t][:], t2[:], rsl(repb, ROWSB, bname), op=OP.add)

    HLN1 = {}; HLN2 = {}
    for t in range(T):
        ln_pair(t, x1_all[:, t * 128:(t + 1) * 128], CEN1, VAR1, 'a')
    for t in range(T):
        ln_rsqrt(t, VAR1, RST1, 'a')
    for t in range(T):
        ln_apply(t, CEN1, RST1, HLN1, 'a', 'g1p', 'b1p')
    for t in range(T):
        sg = sb.tile([P, 128], BF16, tag="sg1")
        nc.scalar.activation(sg[:], HLN1[t][:], ACTF.Sigmoid)
        H1[t] = pht('h1', t, [P, 128])
        nc.gpsimd.tensor_mul(H1[t][:], sg[:], HLN1[t][:])
    for t in range(T):
        hp = ps.tile([P, P], BF16, tag="ps_tp")
        nc.tensor.transpose(hp[:], H1[t][:], ident[:])
        H1T[t] = pht('h1T', t, [P, P])
        nc.scalar.copy(H1T[t][:], hp[:])

    # ============ radial layer 2 ============
    x2_all = psx.tile([P, T * 128], F32, tag="x1_all", name="x2_all")
    for t in range(T):
        nc.tensor.matmul(x2_all[:, t * 128:(t + 1) * 128], H1T[t][:], W2blk[:],
                         start=True, stop=True, skip_group_check=True)
    for t in range(T):
        ln_pair(t, x2_all[:, t * 128:(t + 1) * 128], CEN2, VAR2, 'b')
    for t in range(T):
        ln_rsqrt(t, VAR2, RST2, 'b')
    for t in range(T):
        ln_apply(t, CEN2, RST2, HLN2, 'b', 'g2p', 'b2p')
    for t in range(T):
        sg = sb.tile([P, 128], BF16, tag="sg2")
        nc.scalar.activation(sg[:], HLN2[t][:], ACTF.Sigmoid)
        H2[t] = pht('h2', t, [P, 128])
        nc.gpsimd.tensor_mul(H2[t][:], sg[:], HLN2[t][:])
    for t in range(T):
        hpn = ps.tile([64, P], BF16, tag="ps_tp")
        nc.tensor.transpose(hpn[:], H2[t][:, 0:64], ident[:])
        H2TN[t] = pht('h2Tn', t, [64, P])
        nc.scalar.copy(H2TN[t][:], hpn[:])
        hpe = ps.tile([64, P], BF16, tag="ps_tp")
        nc.tensor.transpose(hpe[:], H2[t][:, 64:128], ident[:])
        H2TE[t] = pht('h2Te', t, [64, P])
        nc.scalar.copy(H2TE[t][:], hpe[:])

    # ==== bubble filler: independent DVE work issued at the radial->dtp
    # boundary (the trace shows ~16 us of DVE idle here waiting on the first
    # chunk's matmul+evac+multiply chain) ====
    OHG = {}; RDEN = {}; OH = {}
    for t in range(T):
        gb = sb.tile([64, P], F32, tag="gidbc")
        nc.gpsimd.partition_broadcast(gb[:], gidr[0:1, tcols(t)])
        OHG[t] = pht('ohg', t, [64, P])
        nc.vector.tensor_tensor(OHG[t][:], ap3(iotap_bf, [[0, P]]), gb[:],
                                op=OP.is_equal)
        den = sb.tile([P, 1], F32, tag="den")
        nc.vector.scalar_tensor_tensor(den[:], ecol(t, 0), 1.0, ecol(t, 0),
                                       op0=OP.add, op1=OP.mult)
        RDEN[t] = pht('rden', t, [P, 1], F32)
        nc.vector.reciprocal(RDEN[t][:], den[:])
        lo, hi = tile_chunks[t]
        for ch in range(lo, hi + 1):
            ssh = sb.tile([P, 1], F32, tag="ssh")
            nc.vector.tensor_scalar_add(ssh[:], ecol(t, 4), float(-P * ch))
            OH[(t, ch)] = pht(f'oh{ch}', t, [P, P])
            nc.vector.tensor_scalar(OH[(t, ch)][:], iota_bf[:], ssh[:, :1],
                                    None, op0=OP.is_equal)

    if STAGE < 5:
        _finish(); return
    # ============ depthwise TP helper ============
    def dtp(t, h2T, W3, s_in, v_in, full, pref):
        nchunks = 20 if full else 10
        bils = pht(f'{pref}bs', t, [P, 64])
        bv0 = pht(f'{pref}v0', t, [P, 96])
        r = {'bil_ss': bils, 'bv0': bv0}
        if full:
            r['bsv'] = pht(f'{pref}sv', t, [P, 192])
            r['bvs'] = pht(f'{pref}vs', t, [P, 32])
            r['cbuf'] = pht(f'{pref}cb', t, [P, 96])
        # shared mult-output buffers: one batched TENSOR_REDUCE per kind
        # amortizes the ~280 ns fixed cost of 8 (or 4) per-chunk reduces
        qall_ss = sbq.tile([P, 4096], BF16, tag="qall_ss",
                           name=f"qall_ss{pref}{t}", bufs=2)
        qall_vs = None
        if full:
            qall_vs = sbq.tile([P, 2048], BF16, tag="qall_vs",
                               name=f"qall_vs{t}", bufs=2)
        for c in range(nchunks):
            pw = psw.tile([P, 512], F32, tag="pw")
            nc.tensor.matmul(pw[:], h2T[:], W3[:, 512 * c:512 * c + 512],
                             start=True, stop=True)
            pwb = sbq.tile([P, 512], BF16, tag="pwb")
            nc.scalar.copy(pwb[:], pw[:])
            if full:
                kind = ('ss' if c < 8 else 'sv' if c < 12 else
                        'vs' if c < 16 else 'v0' if c < 18 else 'v1')
                ci = {'ss': c, 'sv': c - 8, 'vs': c - 12,
                      'v0': c - 16, 'v1': c - 18}[kind]
            else:
                kind = 'ss' if c < 8 else 'v0'
                ci = c if c < 8 else c - 8
            # engine split: GpSimd takes the ss/vs multiplies; DVE the rest.
            if kind in ('ss', 'vs'):
                qdst = qall_ss if kind == 'ss' else qall_vs
                nc.gpsimd.tensor_tensor(
                    ap3(qdst, [[64, 8], [1, 64]], offset=512 * ci),
                    ap3(pwb, [[64, 8], [1, 64]]),
                    ap3(s_in, [[0, 8], [1, 64]]), op=OP.mult)
            else:
                q = sbq.tile([P, 1536], BF16, tag="qv", bufs=2)
                nc.vector.tensor_tensor(
                    ap3(q, [[96, 16], [32, 3], [1, 32]]),
                    ap3(pwb, [[32, 16], [0, 3], [1, 32]]),
                    ap3(v_in, [[0, 16], [32, 3], [1, 32]]), op=OP.mult)
                dst = r['bsv'] if kind == 'sv' else (
                    r['bv0'] if kind == 'v0' else r['cbuf'])
                nc.vector.tensor_reduce(
                    ap3(dst, [[3, 16], [1, 3]], offset=48 * ci),
                    ap3(q, [[96, 16], [32, 3], [1, 32]]), axis=AX.X, op=OP.add)
        nc.vector.tensor_reduce(r['bil_ss'][:, 0:64],
                                ap3(qall_ss, [[64, 64], [1, 64]]),
                                axis=AX.X, op=OP.add)
        if full:
            nc.vector.tensor_reduce(r['bvs'][:, 0:32],
                                    ap3(qall_vs, [[64, 32], [1, 64]]),
                                    axis=AX.X, op=OP.add)
        return r

    # ============ dtp1 + node-fusion ============
    for t in range(T):
        b1 = dtp(t, H2TN[t], W3nf, S2[t][:, :], V2[t][:, :], True, 'n')
        FS[t] = pht('fs', t, [P, 96])
        FV[t] = pht('fv', t, [P, 384])
        fs, fv = FS[t], FV[t]
        nc.vector.scalar_tensor_tensor(fs[:, 0:64], b1['bil_ss'][:], 0.125,
                                       S1[t][:, :], op0=OP.mult, op1=OP.mult)
        t96 = sbq.tile([P, 96], BF16, tag="t96")
        nc.vector.scalar_tensor_tensor(
            ap3(t96, [[3, 32], [1, 3]]),
            ap3(V1[t], [[1, 32], [32, 3]]), 96.0 ** -0.5,
            ap3(b1['bv0'], [[3, 32], [1, 3]]), op0=OP.mult, op1=OP.mult)
        nc.vector.tensor_reduce(fs[:, 64:96], ap3(t96, [[3, 32], [1, 3]]),
                                axis=AX.X, op=OP.add)
        nc.vector.tensor_tensor(fs[:], fs[:], rsl(repb, ROWSB, 'nf_bias'),
                                op=OP.add)
        nc.vector.scalar_tensor_tensor(
            ap3(fv, [[128, 3], [1, 64]]),
            ap3(b1['bsv'], [[1, 3], [3, 64]]), 32.0 ** -0.5,
            ap3(S1[t], [[0, 3], [1, 64]]), op0=OP.mult, op1=OP.mult)
        nc.vector.scalar_tensor_tensor(
            ap3(fv, [[128, 3], [1, 32]], offset=64),
            ap3(V1[t], [[32, 3], [1, 32]]), 0.125,
            ap3(b1['bvs'], [[0, 3], [1, 32]]), op0=OP.mult, op1=OP.mult)
        for x in range(3):
            y, zz = (x + 1) % 3, (x + 2) % 3
            ta = sbq.tile([P, 32], BF16, tag="crossa")
            nc.vector.scalar_tensor_tensor(
                ta[:], V1[t][:, 32 * y:32 * y + 32], 0.125,
                ap3(b1['cbuf'], [[3, 32]], offset=zz), op0=OP.mult, op1=OP.mult)
            tb = sbq.tile([P, 32], BF16, tag="crossb")
            nc.vector.scalar_tensor_tensor(
                tb[:], V1[t][:, 32 * zz:32 * zz + 32], 0.125,
                ap3(b1['cbuf'], [[3, 32]], offset=y), op0=OP.mult, op1=OP.mult)
            nc.gpsimd.tensor_sub(fv[:, 128 * x + 96:128 * x + 128], ta[:], tb[:])

    if STAGE < 6:
        _finish(); return
    for t in range(T):
        fsp = ps.tile([96, P], BF16, tag="ps_tp")
        nc.tensor.transpose(fsp[:], FS[t][:], ident[:])
        fsT = sbq.tile([96, P], BF16, tag="fsT")
        nc.scalar.copy(fsT[:], fsp[:])
        nsp = ps.tile([P, 64], F32, tag="ps_sm")
        nc.tensor.matmul(nsp[:], fsT[:], ntWs[:], start=True, stop=True)
        NS[t] = pht('ns', t, [P, 64])
        nc.vector.tensor_tensor(NS[t][:], nsp[:], rsl(repb, ROWSB, 'nt_bs'),
                                op=OP.add)
        NV[t] = pht('nv', t, [P, 96])
        for x in range(3):
            fvp = ps.tile([P, P], BF16, tag="ps_tp")
            nc.tensor.transpose(fvp[:], FV[t][:, 128 * x:128 * x + 128], ident[:])
            fvT = sbq.tile([P, P], BF16, tag="fvT")
            nc.scalar.copy(fvT[:], fvp[:])
            nvp = ps.tile([P, 32], F32, tag="ps_sm")
            nc.tensor.matmul(nvp[:], fvT[:], ntWv[:], start=True, stop=True)
            nc.scalar.copy(NV[t][:, 32 * x:32 * x + 32], nvp[:])

    if STAGE < 7:
        _finish(); return
    # ============ dtp2 + epilogue2 (fp32 out for adaLN) ============
    for t in range(T):
        b2 = dtp(t, H2TE[t], W3ef, ES[t][:, :], EV[t][:, :], False, 'e')
        AS[t] = pht('as', t, [P, 96], F32)
        as_ = AS[t]
        nc.vector.scalar_tensor_tensor(as_[:, 0:64], b2['bil_ss'][:], 0.125,
                                       NS[t][:, :], op0=OP.mult, op1=OP.mult)
        t96b = sbq.tile([P, 96], BF16, tag="t96b")
        nc.vector.scalar_tensor_tensor(
            ap3(t96b, [[3, 32], [1, 3]]),
            ap3(NV[t], [[1, 32], [32, 3]]), 96.0 ** -0.5,
            ap3(b2['bv0'], [[3, 32], [1, 3]]), op0=OP.mult, op1=OP.mult)
        nc.vector.tensor_reduce(as_[:, 64:96], ap3(t96b, [[3, 32], [1, 3]]),
                                axis=AX.X, op=OP.add)
        nc.vector.tensor_tensor(as_[:], as_[:], rsl(repb, ROWSB, 'ef_bias'),
                                op=OP.add)

    # ============ adaLN ============
    for t in range(T):
        mu = sb.tile([P, 1], F32, tag="amu")
        nc.vector.tensor_reduce(mu[:], AS[t][:], axis=AX.X, op=OP.add)
        nc.vector.tensor_scalar_mul(mu[:], mu[:], 1.0 / S_TP)
        CENA[t] = pht('cena', t, [P, S_TP], F32)
        nc.vector.tensor_scalar(CENA[t][:], AS[t][:], mu[:, :1], None,
                                op0=OP.subtract)
        sq = sb.tile([P, S_TP], F32, tag="asq")
        nc.vector.tensor_mul(sq[:], CENA[t][:], CENA[t][:])
        VARA[t] = pht('vara', t, [P, 1], F32)
        nc.vector.tensor_reduce(VARA[t][:], sq[:], axis=AX.X, op=OP.add)
    for t in range(T):
        stda = pht('stda', t, [P, 1], F32)
        nc.scalar.activation(stda[:], VARA[t][:], ACTF.Sqrt,
                             scale=1.0 / S_TP,
                             bias=repf[:, ROWSF['eps'][0]:ROWSF['eps'][0] + 1])
        RSTA[t] = pht('rsta', t, [P, 1], F32)
        nc.vector.reciprocal(RSTA[t][:], stda[:])
    if STAGE < 8:
        _finish(); return
    # mod gather via one-hot matmul, fused with adaLN apply
    for t in range(T):
        MODPS[t] = ps.tile([P, 192], F32, tag="ps_sm", name=f"modps{t}")
        nc.tensor.matmul(MODPS[t][:], OHG[t][:], modtab[:], start=True,
                         stop=True)
        sn1 = sb.tile([P, S_TP], BF16, tag="sn1")
        nc.vector.scalar_tensor_tensor(sn1[:], CENA[t][:], RSTA[t][:, :1],
                                       MODPS[t][:, S_TP:2 * S_TP],
                                       op0=OP.mult, op1=OP.mult)
        SN[t] = pht('sn', t, [P, S_TP])
        nc.vector.tensor_tensor(SN[t][:], sn1[:], MODPS[t][:, 0:S_TP], op=OP.add)

    # ============ scalar head ============
    for t in range(T):
        snp = ps.tile([96, P], BF16, tag="ps_tp")
        nc.tensor.transpose(snp[:], SN[t][:], ident[:])
        snT = sbq.tile([96, P], BF16, tag="snT")
        nc.scalar.copy(snT[:], snp[:])
        hdp = ps.tile([P, 32], F32, tag="ps_sm")
        nc.tensor.matmul(hdp[:], snT[:], spW1[:], start=True, stop=True)
        HD[t] = pht('hd', t, [P, 32])
        nc.vector.tensor_tensor(HD[t][:], hdp[:], rsl(repb, ROWSB, 'sp_b1'),
                                op=OP.add)
    for t in range(T):
        sg = sb.tile([P, 32], BF16, tag="sg3")
        nc.scalar.activation(sg[:], HD[t][:], ACTF.Sigmoid)
        HDS[t] = pht('hds', t, [P, 32])
        nc.vector.tensor_mul(HDS[t][:], sg[:], HD[t][:])
    for t in range(T):
        swt = sb.tile([P, 32], BF16, tag="swt")
        nc.gpsimd.tensor_tensor(swt[:], HDS[t][:], rsl(repb, ROWSB, 'spW2r'),
                                op=OP.mult)
        swr = sb.tile([P, 1], F32, tag="swr")
        nc.vector.tensor_reduce(swr[:], swt[:], axis=AX.X, op=OP.add)
        sw = sb.tile([P, 1], F32, tag="sw")
        nc.vector.tensor_scalar(sw[:], swr[:], 32.0 ** -0.5,
                                repf[:, ROWSF['sp_b2'][0]:ROWSF['sp_b2'][0] + 1],
                                op0=OP.mult, op1=OP.add)
        coef = sb.tile([P, 1], F32, tag="coef")
        nc.vector.tensor_mul(coef[:], sw[:], RDEN[t][:])
        FORCE[t] = pht('force', t, [P, 3])
        nc.vector.tensor_scalar(FORCE[t][:], edf[:, 8 * t + 1:8 * t + 4],
                                coef[:, :1], None, op0=OP.mult)

    if STAGE < 9:
        _finish(); return
    # ============ scatter (one-hot matmuls over the tile's chunk range) =====
    for t in range(T):
        lo, hi = tile_chunks[t]
        acc_p = ps.tile([P, CHL * 3], F32, tag="ps_sm", name=f"accp{t}")
        for ch in range(lo, hi + 1):
            nc.tensor.matmul(acc_p[:, 3 * ch:3 * ch + 3], OH[(t, ch)][:],
                             FORCE[t][:], start=True, stop=True,
                             skip_group_check=True)
        nc.vector.tensor_add(acc_sb[:, 3 * lo:3 * hi + 3],
                             acc_sb[:, 3 * lo:3 * hi + 3],
                             acc_p[:, 3 * lo:3 * hi + 3])

    if DEBUG:
        for t in range(T):
            e0 = t * P
            dma(Tn['dbg_fs'][e0:e0 + P, :], FS[t][:])
            dma(Tn['dbg_as'][e0:e0 + P, :], AS[t][:])
            dma(Tn['dbg_force'][e0:e0 + P, :], FORCE[t][:])
            dma(Tn['dbg_h2'][e0:e0 + P, :], H2[t][:])
            dma(Tn['dbg_sn'][e0:e0 + P, :], SN[t][:])
            dma(Tn['dbg_fv'][e0:e0 + P, :], FV[t][:])
            dma(Tn['dbg_s1'][e0:e0 + P, :], S1[t][:])
            dma(Tn['dbg_v1'][e0:e0 + P, :], V1[t][:])
            dma(Tn['dbg_es'][e0:e0 + P, :], ES[t][:])
            dma(Tn['dbg_ns'][e0:e0 + P, :], NS[t][:])
            dma(Tn['dbg_nv'][e0:e0 + P, :], NV[t][:])

    # ============ output ============
    _finish()


# ======================= host side =======================

def host_prep(inp):
    inp = {k: np.asarray(v) for k, v in inp.items()}
    src = inp['edge_index'][0].astype(np.int64)
    dst = inp['edge_index'][1].astype(np.int64)
    perm = np.argsort(src, kind='stable')
    src, dst = src[perm], dst[perm]
    gid = inp['batch'].astype(np.int64)[src]
    h_edge = inp['h_edge'][perm]
    dist = inp['distance'][perm].astype(np.float32)
    rvec = inp['relative_vec'][perm].astype(np.float32)
    hn = inp['h_node'].astype(np.float32)

    # scatter geometry
    bases, spans = [], []
    for c in range(NC_CORES):
        sl = src[c * EC:(c + 1) * EC]
        base = int(sl.min()) // P * P
        bases.append(base)
        spans.append(int(sl.max()) - base + 1)
    CHL = max(-(-s // P) for s in spans)
    tile_chunks = []
    for t in range(T):
        lo, hi = CHL, 0
        for c in range(NC_CORES):
            sl = src[c * EC:(c + 1) * EC] - bases[c]
            tl = sl[t * P:(t + 1) * P]
            lo = min(lo, int(tl.min()) // P)
            hi = max(hi, int(tl.max()) // P)
        tile_chunks.append((lo, hi))

    # constant rows
    rf = np.zeros(RWF, np.float32)
    mean = inp['rbf_mean'].astype(np.float32)
    std = inp['rbf_std'].astype(np.float32)
    rw = float(inp['rbf_w']); rb = float(inp['rbf_b'])
    rf[ROWSF['A'][0]:ROWSF['A'][0] + NB] = rw / (CUTOFF * std)
    rf[ROWSF['B'][0]:ROWSF['B'][0] + NB] = (rb - mean) / std
    rf[ROWSF['sp_b2'][0]] = float(inp['sp_b2'][0])
    rf[ROWSF['eps'][0]] = 1e-5

    rbv = np.zeros(RWB, np.float32)

    def setb(name, val):
        off, w = ROWSB[name]
        rbv[off:off + w] = val
    setb('g1p', np.concatenate([inp['nf_g1'], inp['ef_g1']]))
    setb('b1p', np.concatenate([inp['nf_b1'], inp['ef_b1']]))
    setb('g2p', np.concatenate([inp['nf_g2'], inp['ef_g2']]))
    setb('b2p', np.concatenate([inp['nf_b2'], inp['ef_b2']]))
    setb('sbs', inp['src_bs']); setb('dbs', inp['dst_bs'])
    setb('nt_bs', inp['nt_bs']); setb('et_bs', inp['et_bs'])
    setb('nf_bias', inp['nf_bias']); setb('ef_bias', inp['ef_bias'])
    setb('sp_b1', inp['sp_b1']); setb('spW2r', inp['sp_W2'][:, 0])
    nbt = inp['norm_bt'][:2 * S_TP].copy()
    nbt[S_TP:] += 1.0                      # adaLN (1+scale) fold
    setb('normbt', nbt)

    def bf(x):
        return np.ascontiguousarray(np.asarray(x, np.float32).astype(BF))

    W1p = np.concatenate([inp['nf_W1'], inp['ef_W1']], axis=1).astype(np.float32)
    W1p *= (1.0 / (np.sqrt(2 * np.pi) * std))[:, None]
    W2blk = np.zeros((128, 128), np.float32)
    W2blk[:64, :64] = inp['nf_W2']; W2blk[64:, 64:] = inp['ef_W2']
    W3ef = inp['ef_W3']

    def packT(hrows):
        """[n,320] node-feature rows -> [320,n]: scalars then x-major vecs."""
        hs = hrows[:, :128]
        out = [hs.T]
        for x in range(3):
            out.append(hrows[:, 128 + x::3].T)       # [64, n]
        return np.concatenate(out, axis=0)

    def packTe(hrows):
        hs = hrows[:, :64]
        out = [hs.T]
        for x in range(3):
            out.append(hrows[:, 64 + x::3].T)        # [32, n]
        return np.concatenate(out, axis=0)

    shared = dict(
        W3nf=bf(inp['nf_W3']),
        W3ef=bf(np.concatenate([W3ef[:, :4096], W3ef[:, 8192:9216]], axis=1)),
        W1p=bf(W1p), W2blk=bf(W2blk),
        srcWs=bf(inp['src_Ws'] * 128 ** -0.5), dstWs=bf(inp['dst_Ws'] * 128 ** -0.5),
        srcWv=bf(inp['src_Wv'] * 64 ** -0.5), dstWv=bf(inp['dst_Wv'] * 64 ** -0.5),
        ntWs=bf(inp['nt_Ws'] * 96 ** -0.5), ntWv=bf(inp['nt_Wv'] * 128 ** -0.5),
        etWs=bf(inp['et_Ws'] * 64 ** -0.5), etWv=bf(inp['et_Wv'] * 32 ** -0.5),
        spW1=bf(inp['sp_W1'] * 96 ** -0.5),
        normWt=bf(inp['norm_Wt'][:, :2 * S_TP]),
        tT=bf(inp['t'].T),
        rowsf=rf.reshape(1, -1),
        rowsb=bf(rbv.reshape(1, -1)),
    )

    in_maps = []
    for c in range(NC_CORES):
        sl = slice(c * EC, (c + 1) * EC)
        m = dict(shared)
        m['hsT'] = bf(packT(hn[src[sl]]))
        m['hdT'] = bf(packT(hn[dst[sl]]))
        m['heT'] = bf(packTe(h_edge[sl]))
        ed = np.zeros((EC, 8), np.float32)
        ed[:, 0] = dist[sl]
        ed[:, 1:4] = rvec[sl]
        ed[:, 4] = (src[sl] - bases[c]).astype(np.float32)
        m['edf'] = np.ascontiguousarray(
            ed.reshape(T, P, 8).transpose(1, 0, 2).reshape(P, T * 8))
        m['gidr'] = np.ascontiguousarray(
            gid[sl].astype(np.float32).reshape(1, EC))
        in_maps.append(m)
    return in_maps, bases, CHL, tuple(tile_chunks)


_CACHE = {}


def get_nc(CHL, tile_chunks):
    key = (CHL, tile_chunks, STAGE)
    if key not in _CACHE:
        _CACHE[key] = build_nc(CHL, tile_chunks)
    return _CACHE[key]


def kernel(**inputs):
    from concourse.bass_utils import run_bass_kernel_spmd
    in_maps, bases, CHL, tile_chunks = host_prep(inputs)
    nc = get_nc(CHL, tile_chunks)
    res = run_bass_kernel_spmd(nc, in_maps, list(range(NC_CORES)))
    out = np.zeros((N + CHL * P, 3), np.float64)
    for c, r in enumerate(res.results):
        out[bases[c]:bases[c] + CHL * P] += r['outp'].astype(np.float64)
    return out[:N].astype(np.float32)


# revision 44
# speedup vs baseline: 1.2355x; 1.2355x over previous
"""Bass/Trainium2 kernel for nn_EquivariantPosUpdate — 8-core edge-parallel, v2.

Per core: 1024 edges in 8 tiles of 128 (edges on partitions).
Key design vs v1 (1.00 ms -> 0.50 ms on-device):
  - all matmuls in fp16 (fp32 matmul = 4 cy/row + LOW_HIGH double-issue;
    fp16 = 1 cy/row and 8x the mantissa of bf16 -> rel err 1.7e-3)
  - node features gathered per edge on HOST (pure data staging); no phase A,
    no indirect DMAs; all per-edge inputs staged to SBUF in one DMA each
  - radial-MLP stages phased across tiles so the Scalar engine loads each
    activation table once per stage (Exp/Sqrt/Sigmoid): 13 table loads
    total instead of ~70 (1.3 us each)
  - depthwise-TP weight chunks: PE matmul (fp16) -> Scalar evac to fp16 SBUF
    (DVE reads from PSUM are ~3x slower than SBUF) -> ss-multiplies on
    GpSimd, everything else mult+grouped-reduce on DVE (the span limiter)
  - adaLN time-mod table gathered per edge via one-hot matmul (no DRAM trip)
  - scatter: edges sorted by src on host; each core covers a 384-node window;
    per-tile one-hot matmuls only over the 1-3 chunks the tile touches
    (chunk ranges specialized at build time from the actual edge_index)
"""
import sys, os
sys.path.insert(0, '/opt/trn_rl_repo')
import numpy as np
import ml_dtypes
from contextlib import ExitStack

import concourse.bass as bass
import concourse.bacc as bacc
import concourse.mybir as mybir
import concourse.tile as tile
from concourse.bass import AP
from concourse.masks import make_identity

F32 = mybir.dt.float32
BF16 = mybir.dt.float16  # 2-byte; fp16 for precision (same PE/DVE speed)
AX = mybir.AxisListType
OP = mybir.AluOpType
ACTF = mybir.ActivationFunctionType
BF = np.float16

N, E, G, NB = 2048, 8192, 64, 128
NC_CORES = 8
EC = E // NC_CORES          # 1024
P = 128
T = EC // P                 # 8 tiles
M0, M1 = 64, 32
S_TP = 96
CUTOFF = 5.0
DEBUG = False
STAGE = int(os.environ.get('K2STAGE', '99'))

# ---- replicated constant rows ----
ROWSF = {}
_o = 0
for _n, _w in [('A', 128), ('B', 128), ('sp_b2', 1), ('eps', 1)]:
    ROWSF[_n] = (_o, _w); _o += _w
RWF = _o
ROWSB = {}
_o = 0
for _n, _w in [('g1p', 128), ('b1p', 128), ('g2p', 128), ('b2p', 128),
               ('sbs', 64), ('dbs', 64), ('nt_bs', 64), ('et_bs', 64),
               ('nf_bias', 96), ('ef_bias', 96), ('sp_b1', 32),
               ('spW2r', 32), ('normbt', 192)]:
    ROWSB[_n] = (_o, _w); _o += _w
RWB = _o


def rsl(rep, rows, name, nrows=P):
    off, w = rows[name]
    return rep[0:nrows, off:off + w]


def ap3(t, dims, offset=0):
    base = t[:, :] if not isinstance(t, AP) else t
    return AP(base.tensor, base.offset + offset,
              [base.ap[0]] + [list(d) for d in dims])


def build_nc(CHL, tile_chunks):
    """CHL: local node chunks per core; tile_chunks: [(lo,hi)] per tile."""
    nc = bacc.Bacc("TRN2", target_bir_lowering=False, debug=False,
                   num_devices=NC_CORES)
    Tn = {}

    def din(name, shape, dtype=BF16):
        Tn[name] = nc.dram_tensor(name, shape, dtype, kind="ExternalInput")
        return Tn[name]

    din('W3nf', [64, 10240]); din('W3ef', [64, 5120])
    din('W1p', [128, 128]); din('W2blk', [128, 128])
    din('srcWs', [128, 64]); din('dstWs', [128, 64])
    din('srcWv', [64, 32]); din('dstWv', [64, 32])
    din('ntWs', [96, 64]); din('ntWv', [128, 32])
    din('etWs', [64, 64]); din('etWv', [32, 32])
    din('spW1', [96, 32]); din('normWt', [128, 192]); din('tT', [128, G])
    din('hsT', [320, EC]); din('hdT', [320, EC]); din('heT', [160, EC])
    din('rowsf', [1, RWF], F32); din('rowsb', [1, RWB])
    din('edf', [P, T * 8], F32); din('gidr', [1, EC], F32)
    outp = nc.dram_tensor('outp', [CHL * P, 3], F32, kind="ExternalOutput")
    Tn['outp'] = outp
    if DEBUG:
        for nm, sh in [('dbg_fs', [EC, 96]), ('dbg_as', [EC, 96]),
                       ('dbg_force', [EC, 3]), ('dbg_h2', [EC, 128]),
                       ('dbg_sn', [EC, 96]), ('dbg_fv', [EC, 384]),
                       ('dbg_s1', [EC, 64]), ('dbg_v1', [EC, 96]),
                       ('dbg_es', [EC, 64]), ('dbg_ns', [EC, 64]),
                       ('dbg_nv', [EC, 96])]:
            Tn[nm] = nc.dram_tensor(nm, sh, F32, kind="ExternalOutput")

    with tile.TileContext(nc) as tc:
        with ExitStack() as ctx:
            with nc.allow_low_precision(reason="bf16 pipeline; rel-err gate 2e-2"):
                _build(ctx, tc, nc, Tn, CHL, tile_chunks)
    nc.compile()
    return nc


def _build(ctx, tc, nc, Tn, CHL, tile_chunks):
    consts = ctx.enter_context(tc.tile_pool(name="consts", bufs=1))
    ph = ctx.enter_context(tc.tile_pool(name="ph", bufs=1))      # per-tile persist
    sb = ctx.enter_context(tc.tile_pool(name="sb", bufs=4))      # transient
    sbq = ctx.enter_context(tc.tile_pool(name="sbq", bufs=4))    # dtp transient
    ps = ctx.enter_context(tc.tile_pool(name="ps", bufs=2, space="PSUM"))
    psw = ctx.enter_context(tc.tile_pool(name="psw", bufs=2, space="PSUM"))
    psx = ctx.enter_context(tc.tile_pool(name="psx", bufs=1, space="PSUM"))
    dma = nc.sync.dma_start

    def load(name, pool=consts):
        t = pool.tile(Tn[name].shape, Tn[name].dtype, tag="ld_" + name,
                      name="ld_" + name)
        dma(t[:], Tn[name][:])
        return t

    # ---------------- setup ----------------
    # DMA order = need order: per-edge inputs + first-stage weights first,
    # the big W3 tables (only needed ~100us in, at the dtp stage) last.
    edf = load('edf'); gidr = load('gidr')
    rowsf1 = load('rowsf'); rowsb1 = load('rowsb')
    W1p = load('W1p'); W2blk = load('W2blk')
    srcWs = load('srcWs'); dstWs = load('dstWs')
    srcWv = load('srcWv'); dstWv = load('dstWv')
    ntWs = load('ntWs'); ntWv = load('ntWv')
    etWs = load('etWs'); etWv = load('etWv')
    spW1 = load('spW1'); normWt = load('normWt'); tT = load('tT')
    heS = consts.tile([64, EC], BF16)
    dma(heS[:], Tn['heT'][0:64, :])
    heV = [consts.tile([32, EC], BF16, tag=f"heV{x}", name=f"heV{x}")
           for x in range(3)]
    for x in range(3):
        dma(heV[x][:], Tn['heT'][64 + 32 * x:96 + 32 * x, :])
    hsS = consts.tile([128, EC], BF16)
    dma(hsS[:], Tn['hsT'][0:128, :])
    hdS = consts.tile([128, EC], BF16)
    dma(hdS[:], Tn['hdT'][0:128, :])
    hsV = [consts.tile([64, EC], BF16, tag=f"hsV{x}", name=f"hsV{x}")
           for x in range(3)]
    hdV = [consts.tile([64, EC], BF16, tag=f"hdV{x}", name=f"hdV{x}")
           for x in range(3)]
    for x in range(3):
        dma(hsV[x][:], Tn['hsT'][128 + 64 * x:192 + 64 * x, :])
        dma(hdV[x][:], Tn['hdT'][128 + 64 * x:192 + 64 * x, :])
    W3nf = load('W3nf'); W3ef = load('W3ef')

    repf = consts.tile([P, RWF], F32)
    nc.gpsimd.partition_broadcast(repf[:], rowsf1[:])
    repb = consts.tile([P, RWB], BF16)
    nc.gpsimd.partition_broadcast(repb[:], rowsb1[:])

    ident = consts.tile([P, P], BF16)
    make_identity(nc, ident[:])
    iota_i = consts.tile([P, P], mybir.dt.int32)
    nc.gpsimd.iota(iota_i[:], pattern=[[1, P]], base=0, channel_multiplier=0)
    iota_bf = consts.tile([P, P], BF16)
    nc.vector.tensor_copy(iota_bf[:], iota_i[:])
    iotap_i = consts.tile([64, 1], mybir.dt.int32)
    nc.gpsimd.iota(iotap_i[:], pattern=[[1, 1]], base=0, channel_multiplier=1)
    iotap_bf = consts.tile([64, 1], BF16)
    nc.vector.tensor_copy(iotap_bf[:], iotap_i[:])

    # time-mod table [G, 192] = t @ normWt + normbt (scale half has +1 folded)
    md_ps = ps.tile([G, 192], F32, tag="ps_sm")
    nc.tensor.matmul(md_ps[:], tT[:], normWt[:], start=True, stop=True)
    modtab = consts.tile([G, 192], BF16)
    nc.vector.tensor_tensor(modtab[:], md_ps[:], rsl(repb, ROWSB, 'normbt', G),
                            op=OP.add)

    acc_sb = consts.tile([P, CHL * 3], F32)
    nc.vector.memset(acc_sb[:], 0.0)

    # per-tile persistent tiles
    def pht(name, t, shape, dtype=BF16):
        return ph.tile(shape, dtype, tag=f"{name}{t}", name=f"{name}{t}")

    S1 = {}; V1 = {}; S2 = {}; V2 = {}; ES = {}; EV = {}
    ESR = {}; ESRT = {}; CEN1 = {}; RST1 = {}; H1 = {}; H1T = {}
    CEN2 = {}; RST2 = {}; H2 = {}; H2TN = {}; H2TE = {}
    ZSQ = {}; VAR1 = {}; VAR2 = {}; VARA = {}
    FS = {}; FV = {}; NS = {}; NV = {}
    AS = {}; CENA = {}; RSTA = {}; SN = {}; HD = {}; HDS = {}
    FORCE = {}; MODPS = {}

    def tcols(t):
        return slice(t * P, (t + 1) * P)

    def ecol(t, j):
        return edf[:, 8 * t + j:8 * t + j + 1]

    def _finish():
        for ch in range(CHL):
            dma(Tn['outp'][ch * P:(ch + 1) * P, :], acc_sb[:, 3 * ch:3 * ch + 3])

    if STAGE < 2:
        _finish(); return
    # ============ projections: s1/v1 (src), s2/v2 (dst), es/ev (edge) ========
    for t in range(T):
        s1p = ps.tile([P, 64], F32, tag="ps_sm")
        nc.tensor.matmul(s1p[:], hsS[:, tcols(t)], srcWs[:], start=True, stop=True)
        S1[t] = pht('s1', t, [P, 64])
        nc.vector.tensor_tensor(S1[t][:], s1p[:], rsl(repb, ROWSB, 'sbs'), op=OP.add)
        s2p = ps.tile([P, 64], F32, tag="ps_sm")
        nc.tensor.matmul(s2p[:], hdS[:, tcols(t)], dstWs[:], start=True, stop=True)
        S2[t] = pht('s2', t, [P, 64])
        nc.vector.tensor_tensor(S2[t][:], s2p[:], rsl(repb, ROWSB, 'dbs'), op=OP.add)
        V1[t] = pht('v1', t, [P, 96])
        V2[t] = pht('v2', t, [P, 96])
        for x in range(3):
            vp = ps.tile([P, 32], F32, tag="ps_sm")
            nc.tensor.matmul(vp[:], hsV[x][:, tcols(t)], srcWv[:], start=True,
                             stop=True)
            nc.scalar.copy(V1[t][:, 32 * x:32 * x + 32], vp[:])
            vp2 = ps.tile([P, 32], F32, tag="ps_sm")
            nc.tensor.matmul(vp2[:], hdV[x][:, tcols(t)], dstWv[:], start=True,
                             stop=True)
            nc.scalar.copy(V2[t][:, 32 * x:32 * x + 32], vp2[:])
        esp = ps.tile([P, 64], F32, tag="ps_sm")
        nc.tensor.matmul(esp[:], heS[:, tcols(t)], etWs[:], start=True, stop=True)
        ES[t] = pht('es', t, [P, 64])
        nc.vector.tensor_tensor(ES[t][:], esp[:], rsl(repb, ROWSB, 'et_bs'), op=OP.add)
        EV[t] = pht('ev', t, [P, 96])
        for x in range(3):
            evp = ps.tile([P, 32], F32, tag="ps_sm")
            nc.tensor.matmul(evp[:], heV[x][:, tcols(t)], etWv[:], start=True,
                             stop=True)
            nc.scalar.copy(EV[t][:, 32 * x:32 * x + 32], evp[:])

    if STAGE < 3:
        _finish(); return
    # ============ RBF ============
    for t in range(T):
        z = sb.tile([P, NB], F32, tag="z")
        nc.vector.scalar_tensor_tensor(z[:], rsl(repf, ROWSF, 'A'),
                                       ecol(t, 0), rsl(repf, ROWSF, 'B'),
                                       op0=OP.mult, op1=OP.add)
        ZSQ[t] = pht('zsq', t, [P, NB], F32)
        nc.vector.tensor_mul(ZSQ[t][:], z[:], z[:])
    for t in range(T):
        ESR[t] = pht('esr', t, [P, NB])
        nc.scalar.activation(ESR[t][:], ZSQ[t][:], ACTF.Exp, scale=-0.5)
    for t in range(T):
        ep = ps.tile([NB, P], BF16, tag="ps_tp")
        nc.tensor.transpose(ep[:], ESR[t][:], ident[:])
        ESRT[t] = pht('esrT', t, [NB, P])
        nc.scalar.copy(ESRT[t][:], ep[:])

    if STAGE < 4:
        _finish(); return
    # ============ radial layer 1 ============
    x1_all = psx.tile([P, T * 128], F32, tag="x1_all")
    for t in range(T):
        nc.tensor.matmul(x1_all[:, t * 128:(t + 1) * 128], ESRT[t][:], W1p[:],
                         start=True, stop=True, skip_group_check=True)

    def ln_pair(t, x_ps, CEN, VAR, tag):
        """joint LN over two 64-groups; fills CEN/VAR."""
        mu = sb.tile([P, 2], F32, tag=f"mu{tag}")
        nc.vector.tensor_reduce(mu[:], ap3(x_ps, [[64, 2], [1, 64]]),
                                axis=AX.X, op=OP.add)
        nc.vector.tensor_scalar_mul(mu[:], mu[:], 1.0 / 64)
        CEN[t] = pht(f'cen{tag}', t, [P, 128], F32)
        nc.vector.tensor_tensor(CEN[t][:], x_ps, ap3(mu, [[1, 2], [0, 64]]),
                                op=OP.subtract)
        sq = sb.tile([P, 128], F32, tag=f"sq{tag}")
        nc.vector.tensor_mul(sq[:], CEN[t][:], CEN[t][:])
        VAR[t] = pht(f'var{tag}', t, [P, 2], F32)
        nc.vector.tensor_reduce(VAR[t][:], ap3(sq, [[64, 2], [1, 64]]),
                                axis=AX.X, op=OP.add)

    def ln_rsqrt(t, VAR, RST, tag):
        std = pht(f'std{tag}', t, [P, 2], F32)
        nc.scalar.activation(std[:], VAR[t][:], ACTF.Sqrt, scale=1.0 / 64,
                             bias=repf[:, ROWSF['eps'][0]:ROWSF['eps'][0] + 1])
        RST[t] = pht(f'rst{tag}', t, [P, 2], F32)
        nc.vector.reciprocal(RST[t][:], std[:])

    def ln_apply(t, CEN, RST, H, tag, gname, bname):
        t1 = sb.tile([P, 128], BF16, tag=f"t1{tag}")
        nc.vector.tensor_tensor(t1[:], CEN[t][:],
                                ap3(RST[t], [[1, 2], [0, 64]]), op=OP.mult)
        t2 = sb.tile([P, 128], BF16, tag=f"t2{tag}")
        nc.vector.tensor_tensor(t2[:], t1[:], rsl(repb, ROWSB, gname), op=OP.mult)
        H[t] = pht(f'hln{tag}', t, [P, 128])
        nc.vector.tensor_tensor(H[t][:], t2[:], rsl(repb, ROWSB, bname), op=OP.add)

    HLN1 = {}; HLN2 = {}
    for t in range(T):
        ln_pair(t, x1_all[:, t * 128:(t + 1) * 128], CEN1, VAR1, 'a')
    for t in range(T):
        ln_rsqrt(t, VAR1, RST1, 'a')
    for t in range(T):
        ln_apply(t, CEN1, RST1, HLN1, 'a', 'g1p', 'b1p')
    for t in range(T):
        sg = sb.tile([P, 128], BF16, tag="sg1")
        nc.scalar.activation(sg[:], HLN1[t][:], ACTF.Sigmoid)
        H1[t] = pht('h1', t, [P, 128])
        nc.vector.tensor_mul(H1[t][:], sg[:], HLN1[t][:])
    for t in range(T):
        hp = ps.tile([P, P], BF16, tag="ps_tp")
        nc.tensor.transpose(hp[:], H1[t][:], ident[:])
        H1T[t] = pht('h1T', t, [P, P])
        nc.scalar.copy(H1T[t][:], hp[:])

    # ============ radial layer 2 ============
    x2_all = psx.tile([P, T * 128], F32, tag="x1_all", name="x2_all")
    for t in range(T):
        nc.tensor.matmul(x2_all[:, t * 128:(t + 1) * 128], H1T[t][:], W2blk[:],
                         start=True, stop=True, skip_group_check=True)
    for t in range(T):
        ln_pair(t, x2_all[:, t * 128:(t + 1) * 128], CEN2, VAR2, 'b')
    for t in range(T):
        ln_rsqrt(t, VAR2, RST2, 'b')
    for t in range(T):
        ln_apply(t, CEN2, RST2, HLN2, 'b', 'g2p', 'b2p')
    for t in range(T):
        sg = sb.tile([P, 128], BF16, tag="sg2")
        nc.scalar.activation(sg[:], HLN2[t][:], ACTF.Sigmoid)
        H2[t] = pht('h2', t, [P, 128])
        nc.vector.tensor_mul(H2[t][:], sg[:], HLN2[t][:])
    for t in range(T):
        hpn = ps.tile([64, P], BF16, tag="ps_tp")
        nc.tensor.transpose(hpn[:], H2[t][:, 0:64], ident[:])
        H2TN[t] = pht('h2Tn', t, [64, P])
        nc.scalar.copy(H2TN[t][:], hpn[:])
        hpe = ps.tile([64, P], BF16, tag="ps_tp")
        nc.tensor.transpose(hpe[:], H2[t][:, 64:128], ident[:])
        H2TE[t] = pht('h2Te', t, [64, P])
        nc.scalar.copy(H2TE[t][:], hpe[:])

    # ==== bubble filler: independent DVE work issued at the radial->dtp
    # boundary (the trace shows ~16 us of DVE idle here waiting on the first
    # chunk's matmul+evac+multiply chain) ====
    OHG = {}; RDEN = {}; OH = {}
    for t in range(T):
        gb = sb.tile([64, P], F32, tag="gidbc")
        nc.gpsimd.partition_broadcast(gb[:], gidr[0:1, tcols(t)])
        OHG[t] = pht('ohg', t, [64, P])
        nc.vector.tensor_tensor(OHG[t][:], ap3(iotap_bf, [[0, P]]), gb[:],
                                op=OP.is_equal)
        den = sb.tile([P, 1], F32, tag="den")
        nc.vector.scalar_tensor_tensor(den[:], ecol(t, 0), 1.0, ecol(t, 0),
                                       op0=OP.add, op1=OP.mult)
        RDEN[t] = pht('rden', t, [P, 1], F32)
        nc.vector.reciprocal(RDEN[t][:], den[:])
        lo, hi = tile_chunks[t]
        for ch in range(lo, hi + 1):
            ssh = sb.tile([P, 1], F32, tag="ssh")
            nc.vector.tensor_scalar_add(ssh[:], ecol(t, 4), float(-P * ch))
            OH[(t, ch)] = pht(f'oh{ch}', t, [P, P])
            nc.vector.tensor_scalar(OH[(t, ch)][:], iota_bf[:], ssh[:, :1],
                                    None, op0=OP.is_equal)

    if STAGE < 5:
        _finish(); return
    # ============ depthwise TP helper ============
    def dtp(t, h2T, W3, s_in, v_in, full, pref):
        nchunks = 20 if full else 10
        bils = pht(f'{pref}bs', t, [P, 64])
        bv0 = pht(f'{pref}v0', t, [P, 96])
        r = {'bil_ss': bils, 'bv0': bv0}
        if full:
            r['bsv'] = pht(f'{pref}sv', t, [P, 192])
            r['bvs'] = pht(f'{pref}vs', t, [P, 32])
            r['cbuf'] = pht(f'{pref}cb', t, [P, 96])
        # shared mult-output buffers: one batched TENSOR_REDUCE per kind
        # amortizes the ~280 ns fixed cost of 8 (or 4) per-chunk reduces
        qall_ss = sbq.tile([P, 4096], BF16, tag="qall_ss",
                           name=f"qall_ss{pref}{t}", bufs=3)
        qall_vs = None
        if full:
            qall_vs = sbq.tile([P, 2048], BF16, tag="qall_vs",
                               name=f"qall_vs{t}", bufs=2)
        for c in range(nchunks):
            pw = psw.tile([P, 512], F32, tag="pw")
            nc.tensor.matmul(pw[:], h2T[:], W3[:, 512 * c:512 * c + 512],
                             start=True, stop=True)
            pwb = sbq.tile([P, 512], BF16, tag="pwb", bufs=6)
            nc.scalar.copy(pwb[:], pw[:])
            if full:
                kind = ('ss' if c < 8 else 'sv' if c < 12 else
                        'vs' if c < 16 else 'v0' if c < 18 else 'v1')
                ci = {'ss': c, 'sv': c - 8, 'vs': c - 12,
                      'v0': c - 16, 'v1': c - 18}[kind]
            else:
                kind = 'ss' if c < 8 else 'v0'
                ci = c if c < 8 else c - 8
            # engine split: GpSimd takes the ss/vs multiplies; DVE the rest.
            if kind in ('ss', 'vs'):
                qdst = qall_ss if kind == 'ss' else qall_vs
                nc.gpsimd.tensor_tensor(
                    ap3(qdst, [[64, 8], [1, 64]], offset=512 * ci),
                    ap3(pwb, [[64, 8], [1, 64]]),
                    ap3(s_in, [[0, 8], [1, 64]]), op=OP.mult)
            else:
                q = sbq.tile([P, 1536], BF16, tag="qv", bufs=2)
                nc.vector.tensor_tensor(
                    ap3(q, [[96, 16], [32, 3], [1, 32]]),
                    ap3(pwb, [[32, 16], [0, 3], [1, 32]]),
                    ap3(v_in, [[0, 16], [32, 3], [1, 32]]), op=OP.mult)
                dst = r['bsv'] if kind == 'sv' else (
                    r['bv0'] if kind == 'v0' else r['cbuf'])
                nc.vector.tensor_reduce(
                    ap3(dst, [[3, 16], [1, 3]], offset=48 * ci),
                    ap3(q, [[96, 16], [32, 3], [1, 32]]), axis=AX.X, op=OP.add)
        nc.vector.tensor_reduce(r['bil_ss'][:, 0:64],
                                ap3(qall_ss, [[64, 64], [1, 64]]),
                                axis=AX.X, op=OP.add)
        if full:
            nc.vector.tensor_reduce(r['bvs'][:, 0:32],
                                    ap3(qall_vs, [[64, 32], [1, 64]]),
                                    axis=AX.X, op=OP.add)
        return r

    # ============ dtp1 + node-fusion ============
    for t in range(T):
        b1 = dtp(t, H2TN[t], W3nf, S2[t][:, :], V2[t][:, :], True, 'n')
        FS[t] = pht('fs', t, [P, 96])
        FV[t] = pht('fv', t, [P, 384])
        fs, fv = FS[t], FV[t]
        nc.vector.scalar_tensor_tensor(fs[:, 0:64], b1['bil_ss'][:], 0.125,
                                       S1[t][:, :], op0=OP.mult, op1=OP.mult)
        t96 = sbq.tile([P, 96], BF16, tag="t96")
        nc.vector.scalar_tensor_tensor(
            ap3(t96, [[3, 32], [1, 3]]),
            ap3(V1[t], [[1, 32], [32, 3]]), 96.0 ** -0.5,
            ap3(b1['bv0'], [[3, 32], [1, 3]]), op0=OP.mult, op1=OP.mult)
        nc.vector.tensor_reduce(fs[:, 64:96], ap3(t96, [[3, 32], [1, 3]]),
                                axis=AX.X, op=OP.add)
        nc.vector.tensor_tensor(fs[:], fs[:], rsl(repb, ROWSB, 'nf_bias'),
                                op=OP.add)
        nc.vector.scalar_tensor_tensor(
            ap3(fv, [[128, 3], [1, 64]]),
            ap3(b1['bsv'], [[1, 3], [3, 64]]), 32.0 ** -0.5,
            ap3(S1[t], [[0, 3], [1, 64]]), op0=OP.mult, op1=OP.mult)
        nc.vector.scalar_tensor_tensor(
            ap3(fv, [[128, 3], [1, 32]], offset=64),
            ap3(V1[t], [[32, 3], [1, 32]]), 0.125,
            ap3(b1['bvs'], [[0, 3], [1, 32]]), op0=OP.mult, op1=OP.mult)
        for x in range(3):
            y, zz = (x + 1) % 3, (x + 2) % 3
            ta = sbq.tile([P, 32], BF16, tag="crossa")
            nc.vector.scalar_tensor_tensor(
                ta[:], V1[t][:, 32 * y:32 * y + 32], 0.125,
                ap3(b1['cbuf'], [[3, 32]], offset=zz), op0=OP.mult, op1=OP.mult)
            tb = sbq.tile([P, 32], BF16, tag="crossb")
            nc.vector.scalar_tensor_tensor(
                tb[:], V1[t][:, 32 * zz:32 * zz + 32], 0.125,
                ap3(b1['cbuf'], [[3, 32]], offset=y), op0=OP.mult, op1=OP.mult)
            nc.vector.tensor_sub(fv[:, 128 * x + 96:128 * x + 128], ta[:], tb[:])

    if STAGE < 6:
        _finish(); return
    for t in range(T):
        fsp = ps.tile([96, P], BF16, tag="ps_tp")
        nc.tensor.transpose(fsp[:], FS[t][:], ident[:])
        fsT = sbq.tile([96, P], BF16, tag="fsT")
        nc.scalar.copy(fsT[:], fsp[:])
        nsp = ps.tile([P, 64], F32, tag="ps_sm")
        nc.tensor.matmul(nsp[:], fsT[:], ntWs[:], start=True, stop=True)
        NS[t] = pht('ns', t, [P, 64])
        nc.vector.tensor_tensor(NS[t][:], nsp[:], rsl(repb, ROWSB, 'nt_bs'),
                                op=OP.add)
        NV[t] = pht('nv', t, [P, 96])
        for x in range(3):
            fvp = ps.tile([P, P], BF16, tag="ps_tp")
            nc.tensor.transpose(fvp[:], FV[t][:, 128 * x:128 * x + 128], ident[:])
            fvT = sbq.tile([P, P], BF16, tag="fvT")
            nc.scalar.copy(fvT[:], fvp[:])
            nvp = ps.tile([P, 32], F32, tag="ps_sm")
            nc.tensor.matmul(nvp[:], fvT[:], ntWv[:], start=True, stop=True)
            nc.scalar.copy(NV[t][:, 32 * x:32 * x + 32], nvp[:])

    if STAGE < 7:
        _finish(); return
    # ============ dtp2 + epilogue2 (fp32 out for adaLN) ============
    for t in range(T):
        b2 = dtp(t, H2TE[t], W3ef, ES[t][:, :], EV[t][:, :], False, 'e')
        AS[t] = pht('as', t, [P, 96], F32)
        as_ = AS[t]
        nc.vector.scalar_tensor_tensor(as_[:, 0:64], b2['bil_ss'][:], 0.125,
                                       NS[t][:, :], op0=OP.mult, op1=OP.mult)
        t96b = sbq.tile([P, 96], BF16, tag="t96b")
        nc.vector.scalar_tensor_tensor(
            ap3(t96b, [[3, 32], [1, 3]]),
            ap3(NV[t], [[1, 32], [32, 3]]), 96.0 ** -0.5,
            ap3(b2['bv0'], [[3, 32], [1, 3]]), op0=OP.mult, op1=OP.mult)
        nc.vector.tensor_reduce(as_[:, 64:96], ap3(t96b, [[3, 32], [1, 3]]),
                                axis=AX.X, op=OP.add)
        nc.vector.tensor_tensor(as_[:], as_[:], rsl(repb, ROWSB, 'ef_bias'),
                                op=OP.add)

    # ============ adaLN ============
    for t in range(T):
        mu = sb.tile([P, 1], F32, tag="amu")
        nc.vector.tensor_reduce(mu[:], AS[t][:], axis=AX.X, op=OP.add)
        nc.vector.tensor_scalar_mul(mu[:], mu[:], 1.0 / S_TP)
        CENA[t] = pht('cena', t, [P, S_TP], F32)
        nc.vector.tensor_scalar(CENA[t][:], AS[t][:], mu[:, :1], None,
                                op0=OP.subtract)
        sq = sb.tile([P, S_TP], F32, tag="asq")
        nc.vector.tensor_mul(sq[:], CENA[t][:], CENA[t][:])
        VARA[t] = pht('vara', t, [P, 1], F32)
        nc.vector.tensor_reduce(VARA[t][:], sq[:], axis=AX.X, op=OP.add)
    for t in range(T):
        stda = pht('stda', t, [P, 1], F32)
        nc.scalar.activation(stda[:], VARA[t][:], ACTF.Sqrt,
                             scale=1.0 / S_TP,
                             bias=repf[:, ROWSF['eps'][0]:ROWSF['eps'][0] + 1])
        RSTA[t] = pht('rsta', t, [P, 1], F32)
        nc.vector.reciprocal(RSTA[t][:], stda[:])
    if STAGE < 8:
        _finish(); return
    # mod gather via one-hot matmul, fused with adaLN apply
    for t in range(T):
        MODPS[t] = ps.tile([P, 192], F32, tag="ps_sm", name=f"modps{t}")
        nc.tensor.matmul(MODPS[t][:], OHG[t][:], modtab[:], start=True,
                         stop=True)
        sn1 = sb.tile([P, S_TP], BF16, tag="sn1")
        nc.vector.scalar_tensor_tensor(sn1[:], CENA[t][:], RSTA[t][:, :1],
                                       MODPS[t][:, S_TP:2 * S_TP],
                                       op0=OP.mult, op1=OP.mult)
        SN[t] = pht('sn', t, [P, S_TP])
        nc.vector.tensor_tensor(SN[t][:], sn1[:], MODPS[t][:, 0:S_TP], op=OP.add)

    # ============ scalar head ============
    for t in range(T):
        snp = ps.tile([96, P], BF16, tag="ps_tp")
        nc.tensor.transpose(snp[:], SN[t][:], ident[:])
        snT = sbq.tile([96, P], BF16, tag="snT")
        nc.scalar.copy(snT[:], snp[:])
        hdp = ps.tile([P, 32], F32, tag="ps_sm")
        nc.tensor.matmul(hdp[:], snT[:], spW1[:], start=True, stop=True)
        HD[t] = pht('hd', t, [P, 32])
        nc.vector.tensor_tensor(HD[t][:], hdp[:], rsl(repb, ROWSB, 'sp_b1'),
                                op=OP.add)
    for t in range(T):
        sg = sb.tile([P, 32], BF16, tag="sg3")
        nc.scalar.activation(sg[:], HD[t][:], ACTF.Sigmoid)
        HDS[t] = pht('hds', t, [P, 32])
        nc.vector.tensor_mul(HDS[t][:], sg[:], HD[t][:])
    for t in range(T):
        swt = sb.tile([P, 32], BF16, tag="swt")
        nc.vector.tensor_tensor(swt[:], HDS[t][:], rsl(repb, ROWSB, 'spW2r'),
                                op=OP.mult)
        swr = sb.tile([P, 1], F32, tag="swr")
        nc.vector.tensor_reduce(swr[:], swt[:], axis=AX.X, op=OP.add)
        sw = sb.tile([P, 1], F32, tag="sw")
        nc.vector.tensor_scalar(sw[:], swr[:], 32.0 ** -0.5,
                                repf[:, ROWSF['sp_b2'][0]:ROWSF['sp_b2'][0] + 1],
                                op0=OP.mult, op1=OP.add)
        coef = sb.tile([P, 1], F32, tag="coef")
        nc.vector.tensor_mul(coef[:], sw[:], RDEN[t][:])
        FORCE[t] = pht('force', t, [P, 3])
        nc.vector.tensor_scalar(FORCE[t][:], edf[:, 8 * t + 1:8 * t + 4],
                                coef[:, :1], None, op0=OP.mult)

    if STAGE < 9:
        _finish(); return
    # ============ scatter (one-hot matmuls over the tile's chunk range) =====
    for t in range(T):
        lo, hi = tile_chunks[t]
        acc_p = ps.tile([P, CHL * 3], F32, tag="ps_sm", name=f"accp{t}")
        for ch in range(lo, hi + 1):
            nc.tensor.matmul(acc_p[:, 3 * ch:3 * ch + 3], OH[(t, ch)][:],
                             FORCE[t][:], start=True, stop=True,
                             skip_group_check=True)
        nc.vector.tensor_add(acc_sb[:, 3 * lo:3 * hi + 3],
                             acc_sb[:, 3 * lo:3 * hi + 3],
                             acc_p[:, 3 * lo:3 * hi + 3])

    if DEBUG:
        for t in range(T):
            e0 = t * P
            dma(Tn['dbg_fs'][e0:e0 + P, :], FS[t][:])
            dma(Tn['dbg_as'][e0:e0 + P, :], AS[t][:])
            dma(Tn['dbg_force'][e0:e0 + P, :], FORCE[t][:])
            dma(Tn['dbg_h2'][e0:e0 + P, :], H2[t][:])
            dma(Tn['dbg_sn'][e0:e0 + P, :], SN[t][:])
            dma(Tn['dbg_fv'][e0:e0 + P, :], FV[t][:])
            dma(Tn['dbg_s1'][e0:e0 + P, :], S1[t][:])
            dma(Tn['dbg_v1'][e0:e0 + P, :], V1[t][:])
            dma(Tn['dbg_es'][e0:e0 + P, :], ES[t][:])
            dma(Tn['dbg_ns'][e0:e0 + P, :], NS[t][:])
            dma(Tn['dbg_nv'][e0:e0 + P, :], NV[t][:])

    # ============ output ============
    _finish()


# ======================= host side =======================

def host_prep(inp):
    inp = {k: np.asarray(v) for k, v in inp.items()}
    src = inp['edge_index'][0].astype(np.int64)
    dst = inp['edge_index'][1].astype(np.int64)
    perm = np.argsort(src, kind='stable')
    src, dst = src[perm], dst[perm]
    gid = inp['batch'].astype(np.int64)[src]
    h_edge = inp['h_edge'][perm]
    dist = inp['distance'][perm].astype(np.float32)
    rvec = inp['relative_vec'][perm].astype(np.float32)
    hn = inp['h_node'].astype(np.float32)

    # scatter geometry
    bases, spans = [], []
    for c in range(NC_CORES):
        sl = src[c * EC:(c + 1) * EC]
        base = int(sl.min()) // P * P
        bases.append(base)
        spans.append(int(sl.max()) - base + 1)
    CHL = max(-(-s // P) for s in spans)
    tile_chunks = []
    for t in range(T):
        lo, hi = CHL, 0
        for c in range(NC_CORES):
            sl = src[c * EC:(c + 1) * EC] - bases[c]
            tl = sl[t * P:(t + 1) * P]
            lo = min(lo, int(tl.min()) // P)
            hi = max(hi, int(tl.max()) // P)
        tile_chunks.append((lo, hi))

    # constant rows
    rf = np.zeros(RWF, np.float32)
    mean = inp['rbf_mean'].astype(np.float32)
    std = inp['rbf_std'].astype(np.float32)
    rw = float(inp['rbf_w']); rb = float(inp['rbf_b'])
    rf[ROWSF['A'][0]:ROWSF['A'][0] + NB] = rw / (CUTOFF * std)
    rf[ROWSF['B'][0]:ROWSF['B'][0] + NB] = (rb - mean) / std
    rf[ROWSF['sp_b2'][0]] = float(inp['sp_b2'][0])
    rf[ROWSF['eps'][0]] = 1e-5

    rbv = np.zeros(RWB, np.float32)

    def setb(name, val):
        off, w = ROWSB[name]
        rbv[off:off + w] = val
    setb('g1p', np.concatenate([inp['nf_g1'], inp['ef_g1']]))
    setb('b1p', np.concatenate([inp['nf_b1'], inp['ef_b1']]))
    setb('g2p', np.concatenate([inp['nf_g2'], inp['ef_g2']]))
    setb('b2p', np.concatenate([inp['nf_b2'], inp['ef_b2']]))
    setb('sbs', inp['src_bs']); setb('dbs', inp['dst_bs'])
    setb('nt_bs', inp['nt_bs']); setb('et_bs', inp['et_bs'])
    setb('nf_bias', inp['nf_bias']); setb('ef_bias', inp['ef_bias'])
    setb('sp_b1', inp['sp_b1']); setb('spW2r', inp['sp_W2'][:, 0])
    nbt = inp['norm_bt'][:2 * S_TP].copy()
    nbt[S_TP:] += 1.0                      # adaLN (1+scale) fold
    setb('normbt', nbt)

    def bf(x):
        return np.ascontiguousarray(np.asarray(x, np.float32).astype(BF))

    W1p = np.concatenate([inp['nf_W1'], inp['ef_W1']], axis=1).astype(np.float32)
    W1p *= (1.0 / (np.sqrt(2 * np.pi) * std))[:, None]
    W2blk = np.zeros((128, 128), np.float32)
    W2blk[:64, :64] = inp['nf_W2']; W2blk[64:, 64:] = inp['ef_W2']
    W3ef = inp['ef_W3']

    def packT(hrows):
        """[n,320] node-feature rows -> [320,n]: scalars then x-major vecs."""
        hs = hrows[:, :128]
        out = [hs.T]
        for x in range(3):
            out.append(hrows[:, 128 + x::3].T)       # [64, n]
        return np.concatenate(out, axis=0)

    def packTe(hrows):
        hs = hrows[:, :64]
        out = [hs.T]
        for x in range(3):
            out.append(hrows[:, 64 + x::3].T)        # [32, n]
        return np.concatenate(out, axis=0)

    shared = dict(
        W3nf=bf(inp['nf_W3']),
        W3ef=bf(np.concatenate([W3ef[:, :4096], W3ef[:, 8192:9216]], axis=1)),
        W1p=bf(W1p), W2blk=bf(W2blk),
        srcWs=bf(inp['src_Ws'] * 128 ** -0.5), dstWs=bf(inp['dst_Ws'] * 128 ** -0.5),
        srcWv=bf(inp['src_Wv'] * 64 ** -0.5), dstWv=bf(inp['dst_Wv'] * 64 ** -0.5),
        ntWs=bf(inp['nt_Ws'] * 96 ** -0.5), ntWv=bf(inp['nt_Wv'] * 128 ** -0.5),
        etWs=bf(inp['et_Ws'] * 64 ** -0.5), etWv=bf(inp['et_Wv'] * 32 ** -0.5),
        spW1=bf(inp['sp_W1'] * 96 ** -0.5),
        normWt=bf(inp['norm_Wt'][:, :2 * S_TP]),
        tT=bf(inp['t'].T),
        rowsf=rf.reshape(1, -1),
        rowsb=bf(rbv.reshape(1, -1)),
    )

    in_maps = []
    for c in range(NC_CORES):
        sl = slice(c * EC, (c + 1) * EC)
        m = dict(shared)
        m['hsT'] = bf(packT(hn[src[sl]]))
        m['hdT'] = bf(packT(hn[dst[sl]]))
        m['heT'] = bf(packTe(h_edge[sl]))
        ed = np.zeros((EC, 8), np.float32)
        ed[:, 0] = dist[sl]
        ed[:, 1:4] = rvec[sl]
        ed[:, 4] = (src[sl] - bases[c]).astype(np.float32)
        m['edf'] = np.ascontiguousarray(
            ed.reshape(T, P, 8).transpose(1, 0, 2).reshape(P, T * 8))
        m['gidr'] = np.ascontiguousarray(
            gid[sl].astype(np.float32).reshape(1, EC))
        in_maps.append(m)
    return in_maps, bases, CHL, tuple(tile_chunks)


_CACHE = {}


def get_nc(CHL, tile_chunks):
    key = (CHL, tile_chunks, STAGE)
    if key not in _CACHE:
        _CACHE[key] = build_nc(CHL, tile_chunks)
    return _CACHE[key]


def kernel(**inputs):
    from concourse.bass_utils import run_bass_kernel_spmd
    in_maps, bases, CHL, tile_chunks = host_prep(inputs)
    nc = get_nc(CHL, tile_chunks)
    res = run_bass_kernel_spmd(nc, in_maps, list(range(NC_CORES)))
    out = np.zeros((N + CHL * P, 3), np.float64)
    for c, r in enumerate(res.results):
        out[bases[c]:bases[c] + CHL * P] += r['outp'].astype(np.float64)
    return out[:N].astype(np.float32)
